# revision 12
# baseline (speedup 1.0000x reference)
"""Trainium2 Bass kernel for nn_BERTRewardModel (22-layer BERT-style reward
model, B=8 x S=1024, D=768, H=12 heads, alternating global/local-window
attention with RoPE, gated-GELU MLP).

Sharding: pure data-parallel — one sequence per NeuronCore, no collectives.

Device-side layout choices (per core):
  - Residual x kept feature-major [D=768, S=1024] fp32 in SBUF (6 tiles of
    128 partitions).
  - All GEMMs: out = lhsT.T @ rhs with contraction on partitions.
      Q/K/Wo/Wi/Wmo: lhsT = weight tiles (natural [in,out] layout), rhs =
      feature-major activations -> feature-major outputs.
      V: lhsT = activation tiles, rhs = Wv -> token-major V, which is what
      the PV matmul needs as stationary.
  - Scores computed pre-transposed sT[keys, queries] (lhsT = K head tile,
    rhs = Q head block), so softmax normalization constants are per-free-
    column; exp via ScalarE from PSUM; denominator comes free as row 64 of
    the PV output via a ones-column appended to V.
  - Local layers only compute the 256-wide query window per key tile
    (banded attention), with a compile-time band mask multiply.
  - RoPE: rot-half built with 4 partition-shift SBUF->SBUF DMAs, sign and
    the 1/sqrt(hd) query scale folded into host-precomputed cos/sin tables.
  - LayerNorm stats via ones-column matmuls in float32r over fp32 x and
    ACT-squared x; rstd = exp(-0.5*ln(var+eps)) on ScalarE; normalization
    applied in bf16 with gpsimd partition-broadcast stats.
  - Norm weights folded into the following weight matrices on the host;
    weights cast to bf16 on the host and streamed HBM->SBUF per layer.

Host side: embedding gather + embedding LayerNorm, final pooled head
(logits = pooled @ cls_w + b, sigmoid) in numpy.
"""
import numpy as np
import ml_dtypes
from contextlib import ExitStack

import concourse.bass as bass
import concourse.bacc as bacc
import concourse.mybir as mybir
import concourse.tile as tile
from concourse.bass_utils import run_bass_kernel_spmd

F32 = mybir.dt.float32
F32R = mybir.dt.float32r
BF16 = mybir.dt.bfloat16
AF = mybir.ActivationFunctionType
ALU = mybir.AluOpType
BF = ml_dtypes.bfloat16

# model dims
D, H, HD, I = 768, 12, 64, 1152
DKT = D // 128            # 6 feature tiles
IKT = I // 128            # 9
EPS = 1e-5
WIN = 128                 # local attention total window width


def default_cfg():
    return dict(S=1024, layers=[(l, l % 3 == 0) for l in range(22)])


# ---------------------------------------------------------------------------
# host-side weight prep
# ---------------------------------------------------------------------------

def rope_tables(theta, S):
    inv = 1.0 / (theta ** (np.arange(0, HD, 2, dtype=np.float32) / HD))  # [32]
    f = np.arange(S, dtype=np.float32)[:, None] * inv[None, :]           # [S, 32]
    cosf = np.cos(f).T.astype(np.float32)    # [32, S]
    sinf = np.sin(f).T.astype(np.float32)    # [32, S]
    # table rows r in [0,128): feature dim d = r % 64, freq idx = d % 32,
    # sign for the sin table: rows with d < 32 multiply rot values (-x2).
    cos_t = np.tile(cosf, (4, 1))                                 # [128, S]
    sin_t = np.concatenate([-sinf, sinf, -sinf, sinf], axis=0)    # [128, S]
    return cos_t, sin_t


def prep_tables(S):
    """tabs tensor [5, 128, S] bf16: cos_g, sin_g, cos_l, sin_l, maskL.
    The 1/sqrt(HD) score scale is applied inside the exp activation."""
    tabs = np.zeros((5, 128, S), dtype=np.float32)
    cg, sg = rope_tables(160000.0, S)
    cl, sl = rope_tables(10000.0, S)
    tabs[0], tabs[1] = cg, sg
    tabs[2], tabs[3] = cl, sl
    # local band mask [128 keys-in-tile, 256 window-cols]: 1 if c-128<=p<=c
    p = np.arange(128)[:, None]
    c = np.arange(256)[None, :]
    tabs[4, :, 0:256] = ((c - 128 <= p) & (p <= c)).astype(np.float32)
    return tabs.astype(BF)


def prep_weights(layers, Wqkv, Wo, Wi, Wmo, attn_norm_w, mlp_norm_w):
    """Flat bf16 blob per layer: wq,wk,wv,wo [D,D] each, wi [D,2I], wmo [I,D].
    Norm weights folded into the rows of the consuming matrices."""
    chunks = []
    for (l, _g) in layers:
        anw = attn_norm_w[l][:, None].astype(np.float32)
        mnw = mlp_norm_w[l][:, None].astype(np.float32)
        if l == 0:
            anw = np.ones_like(anw)  # layer 0 skips attn LN entirely
        wq = Wqkv[l][:, 0:D] * anw
        wk = Wqkv[l][:, D:2 * D] * anw
        wv = Wqkv[l][:, 2 * D:3 * D] * anw
        wo = Wo[l]
        wi = Wi[l] * mnw
        wmo = Wmo[l]
        for a in (wq, wk, wv, wo, wi, wmo):
            chunks.append(np.ascontiguousarray(a, dtype=np.float32).reshape(-1))
    blob = np.concatenate(chunks).astype(BF)
    return blob


LAYER_ELEMS = D * D * 4 + D * 2 * I + I * D
W_OFF = dict(wq=0, wk=D * D, wv=2 * D * D, wo=3 * D * D,
             wi=4 * D * D, wmo=4 * D * D + 2 * D * I)


# ---------------------------------------------------------------------------
# device kernel builder
# ---------------------------------------------------------------------------

def build_model(cfg):
    S = cfg["S"]
    T = S // 128              # token tiles
    layers = cfg["layers"]
    NL = len(layers)

    nc = bacc.Bacc(None, target_bir_lowering=False)
    x0_d = nc.declare_dram_parameter("x0", [D, S], F32, isOutput=False)
    wb_d = nc.declare_dram_parameter("wblob", [NL * LAYER_ELEMS], BF16,
                                     isOutput=False)
    tb_d = nc.declare_dram_parameter("tabs", [5, 128, S], BF16, isOutput=False)
    pooled_d = nc.declare_dram_parameter("pooled", [D], F32, isOutput=True)

    with tile.TileContext(nc) as tc:
        with ExitStack() as ctx:
            build_body(ctx, tc, nc, cfg, x0_d, wb_d, tb_d, pooled_d)
    nc.finalize()
    return nc


def build_body(ctx, tc, nc, cfg, x0_d, wb_d, tb_d, pooled_d):
    S = cfg["S"]
    T = S // 128
    layers = cfg["layers"]
    SLAB = min(512, S)
    NSLAB = (S + SLAB - 1) // SLAB

    persist = ctx.enter_context(tc.tile_pool(name="persist", bufs=1))
    act = ctx.enter_context(tc.tile_pool(name="act", bufs=1))
    tmp = ctx.enter_context(tc.tile_pool(name="tmp", bufs=2))
    wpool = ctx.enter_context(tc.tile_pool(name="wpool", bufs=2))
    ptpool = ctx.enter_context(tc.tile_pool(name="ptpool", bufs=2))
    denpool = ctx.enter_context(tc.tile_pool(name="denpool", bufs=1))
    drb = ctx.enter_context(tc.tile_pool(name="drb", bufs=2, space="DRAM"))
    psA = ctx.enter_context(tc.tile_pool(name="psA", bufs=2, space="PSUM"))
    psB = ctx.enter_context(tc.tile_pool(name="psB", bufs=1, space="PSUM"))
    psS = ctx.enter_context(tc.tile_pool(name="psS", bufs=2, space="PSUM"))

    # --- persistent tiles
    x_res = persist.tile([128, DKT, S], F32, tag="x_res")
    tabs = persist.tile([128, 5, S], BF16, tag="tabs")
    zeros_bf = persist.tile([128, 512], BF16, tag="zeros")
    ones_bf = persist.tile([128, 1], BF16, tag="ones")
    eps_t = persist.tile([128, 1], F32, tag="eps")
    v_aug = persist.tile([128, T, H * 65], BF16, tag="v_aug")

    nc.vector.memset(zeros_bf, 0.0)
    nc.vector.memset(ones_bf, 1.0)
    nc.vector.memset(eps_t, EPS)
    # ones columns of v_aug (written once; V copies never touch them)
    nc.vector.memset(
        v_aug[:].rearrange("p t (h c) -> p t h c", h=H)[:, :, :, 64:65], 1.0)

    # x0 load
    x0r = x0_d[:, :].rearrange("(kt p) s -> kt p s", p=128)
    for kt in range(DKT):
        nc.sync.dma_start(x_res[:, kt, :], x0r[kt])
    # tables load
    for i in range(5):
        nc.sync.dma_start(tabs[:, i, :], tb_d[i])

    def tab(i):
        return tabs[:, i, :]

    def wdma(l, name, shape):
        """DMA a weight chunk [rows, cols] -> sbuf [128, rows//128, cols]."""
        rows, cols = shape
        kts = rows // 128
        t = wpool.tile([128, kts, cols], BF16, tag="w")
        off = l * LAYER_ELEMS + W_OFF[name]
        src = bass.AP(tensor=wb_d, offset=off,
                      ap=[[cols, 128], [128 * cols, kts], [1, cols]])
        nc.sync.dma_start(t[:], src)
        return t

    def wdma_cols(l, name, rows, col0, col1, ncols_total):
        """DMA a column slice of a weight chunk."""
        kts = rows // 128
        cols = col1 - col0
        t = wpool.tile([128, kts, cols], BF16, tag="w")
        off = l * LAYER_ELEMS + W_OFF[name] + col0
        src = bass.AP(tensor=wb_d, offset=off,
                      ap=[[ncols_total, 128], [128 * ncols_total, kts],
                          [1, cols]])
        nc.sync.dma_start(t[:], src)
        return t

    # ---------- LayerNorm (feature-major) ----------
    def layernorm(x_in, out_bf16_tile, name):
        """x_in: [128, DKT, S] fp32. Writes normalized bf16 (no weight) into
        out_bf16_tile [128, DKT, S]."""
        mean_b = tmp.tile([128, S], BF16, tag="mean_b")
        rstd_b = tmp.tile([128, S], BF16, tag="rstd_b")
        stat_f = denpool.tile([1, 2, S], F32, tag="stat")
        stat_bf = denpool.tile([1, 2, S], BF16, tag="statbf")
        for half in range(NSLAB):
            sl = slice(half * SLAB, (half + 1) * SLAB)
            su = psS.tile([1, SLAB], F32, tag="stat")
            sq = psS.tile([1, SLAB], F32, tag="stat")
            for kt in range(DKT):
                xbf = tmp.tile([128, SLAB], BF16, tag="xbfs")
                nc.vector.tensor_copy(xbf, x_in[:, kt, sl])
                nc.tensor.matmul(su, ones_bf, xbf[:],
                                 start=(kt == 0), stop=(kt == DKT - 1))
                xsq = tmp.tile([128, SLAB], BF16, tag="xsq")
                nc.scalar.activation(xsq, x_in[:, kt, sl], AF.Square)
                nc.tensor.matmul(sq, ones_bf, xsq[:],
                                 start=(kt == 0), stop=(kt == DKT - 1))
            # mean = su/D ; var = sq/D - mean^2
            mrow = stat_f[0:1, 0, sl]
            vrow = stat_f[0:1, 1, sl]
            nc.vector.tensor_scalar_mul(mrow, su, 1.0 / D)
            msq = denpool.tile([1, SLAB], F32, tag="msq")
            nc.vector.tensor_mul(msq, mrow, mrow)
            nc.vector.scalar_tensor_tensor(
                out=vrow, in0=sq, scalar=1.0 / D, in1=msq,
                op0=ALU.mult, op1=ALU.subtract)
            # rstd = exp(-0.5 * ln(var + eps))
            nc.scalar.activation(vrow, vrow, AF.Ln, bias=eps_t[0:1])
            nc.scalar.activation(vrow, vrow, AF.Exp, scale=-0.5)
            nc.vector.tensor_copy(stat_bf[0:1, 0, sl], mrow)
            nc.vector.tensor_copy(stat_bf[0:1, 1, sl], vrow)
        scr = drb.tile([2, S], BF16, tag="scr_stat")
        nc.sync.dma_start(scr[:], stat_bf[0:1, :, :])
        nc.sync.dma_start(
            mean_b[:], bass.AP(tensor=scr.tensor, offset=scr.offset,
                               ap=[[0, 128], [1, S]]))
        nc.sync.dma_start(
            rstd_b[:], bass.AP(tensor=scr.tensor, offset=scr.offset + S,
                               ap=[[0, 128], [1, S]]))
        for kt in range(DKT):
            xbf = tmp.tile([128, S], BF16, tag="xbf")
            nc.vector.tensor_copy(xbf, x_in[:, kt, :])
            nc.vector.tensor_sub(xbf, xbf, mean_b)
            nc.vector.tensor_mul(out_bf16_tile[:, kt, :], xbf, rstd_b)

    # ---------- per layer ----------
    for li, (l, is_glob) in enumerate(layers):
        # ---- LN1 / cast
        h = act.tile([128, DKT, S], BF16, tag="hbuf")
        if l == 0:
            for kt in range(DKT):
                nc.vector.tensor_copy(h[:, kt, :], x_res[:, kt, :])
        else:
            layernorm(x_res, h, f"ln1_{l}")

        wq = wdma(li, "wq", (D, D))
        wk = wdma(li, "wk", (D, D))
        wv = wdma(li, "wv", (D, D))

        ct, st = (tab(0), tab(1)) if is_glob else (tab(2), tab(3))

        # ---- Q and K with RoPE
        q_rope = act.tile([128, DKT, S], BF16, tag="q_rope")
        k_rope = act.tile([128, DKT, S], BF16, tag="k_rope")
        for (wmat, dst) in ((wq, q_rope), (wk, k_rope)):
            for mt in range(DKT):
                ps = psA.tile([128, S], F32, tag="mm")
                for kt in range(DKT):
                    for half in range(NSLAB):
                        sl = slice(half * SLAB, (half + 1) * SLAB)
                        nc.tensor.matmul(
                            ps[:, sl], wmat[:, kt, mt * 128:(mt + 1) * 128],
                            h[:, kt, sl], start=(kt == 0), stop=(kt == DKT - 1))
                qbf = tmp.tile([128, S], BF16, tag="qbf")
                nc.scalar.activation(qbf, ps, AF.Copy)
                rot = tmp.tile([128, S], BF16, tag="rot")
                for blk in range(4):
                    sp = (blk ^ 1) * 32   # src partition block (swap pairs)
                    dp = blk * 32
                    nc.sync.dma_start(rot[dp:dp + 32, :], qbf[sp:sp + 32, :])
                t1 = tmp.tile([128, S], BF16, tag="t1")
                nc.vector.tensor_mul(t1, qbf, ct)
                nc.vector.tensor_mul(rot, rot, st)
                nc.vector.tensor_add(dst[:, mt, :], t1, rot)

        # ---- V (token-major: lhsT = h tiles, rhs = wv)
        for tt in range(T):
            ps = psA.tile([128, D], F32, tag="mm")
            for kt in range(DKT):
                for (a, b) in ((0, 512), (512, 768)):
                    nc.tensor.matmul(ps[:, a:b],
                                     h[:, kt, tt * 128:(tt + 1) * 128],
                                     wv[:, kt, a:b],
                                     start=(kt == 0), stop=(kt == DKT - 1))
            dst = v_aug[:].rearrange("p t (h c) -> p t h c", h=H)[
                :, tt, :, 0:64]
            nc.vector.tensor_copy(
                dst, ps[:].rearrange("p (h c) -> p h c", h=H))

        wo = wdma(li, "wo", (D, D))

        # ---- attention
        attn = act.tile([128, DKT, S], BF16, tag="hbuf")
        for hh in range(H):
            ht, hr = hh // 2, (hh % 2) * 64
            qh = q_rope[hr:hr + 64, ht, :]
            kh = k_rope[hr:hr + 64, ht, :]
            apsum = psB.tile([65, S], F32, tag="acc")
            vh = v_aug[:].rearrange("p t (h c) -> p t h c", h=H)
            if not is_glob:
                for half in range(NSLAB):
                    nc.tensor.matmul(
                        apsum[:, half * SLAB:(half + 1) * SLAB],
                        vh[:, 0, hh, :], zeros_bf[:, 0:SLAB],
                        start=True, stop=False, skip_group_check=True)
            for kt in range(T):
                if is_glob:
                    sps = psA.tile([128, S], F32, tag="mm")
                    for half in range(NSLAB):
                        sl = slice(half * SLAB, (half + 1) * SLAB)
                        nc.tensor.matmul(sps[:, sl],
                                         kh[:, kt * 128:(kt + 1) * 128],
                                         qh[:, sl], start=True, stop=True)
                    pt = ptpool.tile([128, S], BF16, tag="pt")
                    nc.scalar.activation(pt, sps, AF.Exp,
                                         scale=1.0 / float(np.sqrt(HD)))
                    for half in range(NSLAB):
                        sl = slice(half * SLAB, (half + 1) * SLAB)
                        nc.tensor.matmul(apsum[:, sl], vh[:, kt, hh, :],
                                         pt[:, sl], start=(kt == 0),
                                         stop=(kt == T - 1))
                else:
                    o = 128 * kt - WIN // 2
                    qlo, qhi = max(0, o), min(S, o + 256)
                    c0, c1 = qlo - o, qhi - o
                    sps = psA.tile([128, 256], F32, tag="mm")
                    nc.tensor.matmul(sps[:, c0:c1],
                                     kh[:, kt * 128:(kt + 1) * 128],
                                     qh[:, qlo:qhi], start=True, stop=True)
                    pt = ptpool.tile([128, 256], BF16, tag="pt")
                    nc.scalar.activation(pt[:, c0:c1], sps[:, c0:c1], AF.Exp,
                                         scale=1.0 / float(np.sqrt(HD)))
                    nc.vector.tensor_mul(pt[:, c0:c1], pt[:, c0:c1],
                                         tab(4)[:, c0:c1])
                    # PV over the window, split at psum bank boundary
                    bsplit = [b for b in range(SLAB, S, SLAB) if qlo < b < qhi]
                    segs = [qlo] + bsplit + [qhi]
                    for si in range(len(segs) - 1):
                        a, b = segs[si], segs[si + 1]
                        nc.tensor.matmul(apsum[:, a:b], vh[:, kt, hh, :],
                                         pt[:, a - o:b - o],
                                         start=False,
                                         stop=(kt == T - 1 and
                                               si == len(segs) - 2),
                                         skip_group_check=True)
            # normalize: 1/den via ACT exp(-ln(den)), DRAM-bounce broadcast
            den = denpool.tile([65, S], F32, tag="den")
            rec = denpool.tile([65, S], BF16, tag="rec")
            nc.scalar.activation(den[64:65, :], apsum[64:65, :], AF.Ln)
            nc.scalar.activation(rec[64:65, :], den[64:65, :], AF.Exp,
                                 scale=-1.0)
            scr = drb.tile([1, S], BF16, tag="scr_rec")
            nc.sync.dma_start(scr[:], rec[64:65, :])
            rec_b = tmp.tile([64, S], BF16, tag="rec_b")
            nc.sync.dma_start(
                rec_b[:], bass.AP(tensor=scr.tensor, offset=scr.offset,
                                  ap=[[0, 64], [1, S]]))
            nc.vector.tensor_mul(attn[hr:hr + 64, ht, :], apsum[0:64, :],
                                 rec_b)

        # ---- Wo + residual
        for mt in range(DKT):
            ps = psA.tile([128, S], F32, tag="mm")
            for kt in range(DKT):
                for half in range(NSLAB):
                    sl = slice(half * SLAB, (half + 1) * SLAB)
                    nc.tensor.matmul(ps[:, sl],
                                     wo[:, kt, mt * 128:(mt + 1) * 128],
                                     attn[:, kt, sl],
                                     start=(kt == 0), stop=(kt == DKT - 1))
            nc.vector.tensor_add(x_res[:, mt, :], ps, x_res[:, mt, :])

        # ---- MLP
        hm = act.tile([128, DKT, S], BF16, tag="hbuf")
        layernorm(x_res, hm, f"ln2_{l}")
        wi1 = wdma_cols(li, "wi", D, 0, I, 2 * I)
        wi2 = wdma_cols(li, "wi", D, I, 2 * I, 2 * I)
        wmo = wdma(li, "wmo", (I, D))
        gated = act.tile([128, IKT, S], BF16, tag="gated")
        for mt in range(IKT):
            ps_i = psA.tile([128, S], F32, tag="mm")
            ps_g = psB.tile([128, S], F32, tag="acc")
            for kt in range(DKT):
                for half in range(NSLAB):
                    sl = slice(half * SLAB, (half + 1) * SLAB)
                    nc.tensor.matmul(ps_i[:, sl],
                                     wi1[:, kt, mt * 128:(mt + 1) * 128],
                                     hm[:, kt, sl],
                                     start=(kt == 0), stop=(kt == DKT - 1))
            for kt in range(DKT):
                for half in range(NSLAB):
                    sl = slice(half * SLAB, (half + 1) * SLAB)
                    nc.tensor.matmul(ps_g[:, sl],
                                     wi2[:, kt, mt * 128:(mt + 1) * 128],
                                     hm[:, kt, sl],
                                     start=(kt == 0), stop=(kt == DKT - 1))
            gl = tmp.tile([128, S], BF16, tag="gelu")
            nc.scalar.activation(gl, ps_i, AF.Gelu)
            nc.vector.tensor_mul(gated[:, mt, :], ps_g, gl)
        for mt in range(DKT):
            ps = psA.tile([128, S], F32, tag="mm")
            for kt in range(IKT):
                for half in range(NSLAB):
                    sl = slice(half * SLAB, (half + 1) * SLAB)
                    nc.tensor.matmul(ps[:, sl],
                                     wmo[:, kt, mt * 128:(mt + 1) * 128],
                                     gated[:, kt, sl],
                                     start=(kt == 0), stop=(kt == IKT - 1))
            nc.vector.tensor_add(x_res[:, mt, :], ps, x_res[:, mt, :])

    # ---- final LN + pooling
    hf = act.tile([128, DKT, S], BF16, tag="hbuf")
    layernorm(x_res, hf, "lnf")
    pooled_sb = persist.tile([128, DKT], F32, tag="pooled")
    for kt in range(DKT):
        nc.vector.reduce_sum(pooled_sb[:, kt:kt + 1], hf[:, kt, :],
                             axis=mybir.AxisListType.X)
    dst = bass.AP(tensor=pooled_d, offset=0,
                  ap=[[1, 128], [128, DKT]])
    nc.sync.dma_start(dst, pooled_sb[:])


# ---------------------------------------------------------------------------
# host wrapper
# ---------------------------------------------------------------------------

def host_pre(cfg, input_ids, tok_emb, emb_norm_w):
    """Returns x0 [B, D, S] fp32: emb lookup + emb LayerNorm, feature-major."""
    x = tok_emb[input_ids]                      # [B, S, D] fp32
    x = x.astype(np.float32)
    m = x.mean(-1, keepdims=True)
    v = x.var(-1, keepdims=True)
    x = (x - m) / np.sqrt(v + EPS) * emb_norm_w
    return np.ascontiguousarray(np.transpose(x, (0, 2, 1)))


def host_post(pooled, S, final_norm_w, cls_w, cls_b, attention_mask):
    """pooled: [B, D] sums over tokens of (x-mu)*rstd."""
    denom = np.maximum(attention_mask.sum(-1, keepdims=True), 1e-9)
    p = pooled / denom.astype(np.float32)
    logits = p @ (final_norm_w[:, None] * cls_w) + cls_b
    scores = 1.0 / (1.0 + np.exp(-logits))
    return scores[:, 0].astype(np.float32), logits.astype(np.float32)


_CACHED = {}


def make_run_args(input_ids, attention_mask, tok_emb, emb_norm_w,
                  attn_norm_w, Wqkv, Wo, mlp_norm_w, Wi, Wmo, final_norm_w,
                  cls_w, cls_b):
    cfg = default_cfg()
    S = cfg["S"]
    input_ids = np.asarray(input_ids)
    attention_mask = np.asarray(attention_mask)
    B = input_ids.shape[0]

    x0 = host_pre(cfg, input_ids, np.asarray(tok_emb, np.float32),
                  np.asarray(emb_norm_w, np.float32))
    blob = prep_weights(cfg["layers"], np.asarray(Wqkv, np.float32),
                        np.asarray(Wo, np.float32), np.asarray(Wi, np.float32),
                        np.asarray(Wmo, np.float32),
                        np.asarray(attn_norm_w, np.float32),
                        np.asarray(mlp_norm_w, np.float32))
    tabs = prep_tables(S)
    nc = build_model(cfg)
    in_maps = [{"x0": x0[c], "wblob": blob, "tabs": tabs} for c in range(B)]
    post = dict(S=S, final_norm_w=np.asarray(final_norm_w, np.float32),
                cls_w=np.asarray(cls_w, np.float32),
                cls_b=np.asarray(cls_b, np.float32),
                attention_mask=attention_mask)
    return cfg, nc, in_maps, post


def kernel(input_ids, attention_mask, tok_emb, emb_norm_w, attn_norm_w, Wqkv,
           Wo, mlp_norm_w, Wi, Wmo, final_norm_w, cls_w, cls_b):
    cfg, nc, in_maps, post = make_run_args(
        input_ids, attention_mask, tok_emb, emb_norm_w, attn_norm_w, Wqkv,
        Wo, mlp_norm_w, Wi, Wmo, final_norm_w, cls_w, cls_b)
    B = len(in_maps)
    res = run_bass_kernel_spmd(nc, in_maps, list(range(B)))
    pooled = np.stack([res.results[c]["pooled"] for c in range(B)])
    return host_post(pooled, post["S"], post["final_norm_w"], post["cls_w"],
                     post["cls_b"], post["attention_mask"])
